# revision 4
# baseline (speedup 1.0000x reference)
"""BitNetLinear forward on 8 TRN2 NeuronCores.

out = x @ (alpha * clip(round(W/alpha), -1, 1))^T
  x [4, 2048, 4096] f32, W [4096, 4096] f32, alpha scalar f32.

Strategy: data-parallel over the 8192 x-rows (1024 rows/core), W replicated.
No collectives. Host side only reshapes/slices (layout); all arithmetic
(ternary quantization + matmul + alpha scaling) runs on device.

Device kernel (per core):
  - x^T shard resident in SBUF as bf16 (4 tiles of [128, 32, 256]).
  - W^T streamed in [128, 4, 512] f32 chunks; ternarized on the fly via
    T' = Sign(w + a/2) + Sign(w - a/2) in {-2, 0, 2}  (bit-exact vs
    clip(round(w/a)) up to measure-zero ties; validated on HW).
  - 2048 matmuls (bf16, FD=512) accumulate into 8 rotating PSUM banks.
  - PSUM evicted through ScalarE with scale = alpha/2 (folds both the
    ternary doubling and the alpha weight scale), DMA to out.
"""

import contextlib
import sys

if "/opt/trn_rl_repo" not in sys.path:
    sys.path.insert(0, "/opt/trn_rl_repo")

import numpy as np

import concourse.bass as bass  # noqa: F401
import concourse.mybir as mybir
import concourse.tile as tile
from concourse import bacc
from concourse.bass_utils import run_bass_kernel_spmd

P = 128
N_CORES = 8
D_IN = 4096  # contraction
D_OUT = 4096
M_TOT = 4 * 2048
M_SHARD = M_TOT // N_CORES  # 1024
KO = D_IN // P  # 32 k-tiles
N_TILE = 512
XG = 4  # x resident tiles (m column groups)

F32 = mybir.dt.float32
BF16 = mybir.dt.bfloat16


def build(m_shard=M_SHARD, d_in=D_IN, d_out=D_OUT, reps=1):
    ko = d_in // P
    n_tiles = d_out // N_TILE
    m_sub = m_shard // P
    xg = min(XG, max(1, m_shard // 256))
    xw = m_shard // xg

    nc = bacc.Bacc("TRN2", target_bir_lowering=False, debug=False,
                   num_devices=N_CORES)
    xt_d = nc.declare_dram_parameter("xt", [P, ko, m_shard], F32, isOutput=False)
    wt_d = nc.declare_dram_parameter("wt", [P, ko, d_out], F32, isOutput=False)
    al_d = nc.declare_dram_parameter("alpha", [1, 1], F32, isOutput=False)
    out_d = nc.declare_dram_parameter("out", [P, m_sub, d_out], F32, isOutput=True)

    with tile.TileContext(nc) as tc:
        with (
            tc.tile_pool(name="const", bufs=1) as const,
            tc.tile_pool(name="xres", bufs=1) as xres_pool,
            tc.tile_pool(name="stage", bufs=4) as stage,
            tc.tile_pool(name="wq", bufs=16) as wqp,
            tc.tile_pool(name="s2", bufs=2) as s2p,
            tc.tile_pool(name="outs", bufs=4) as outs,
            tc.tile_pool(name="psum", bufs=8, space="PSUM") as psum,
        ):
            rep_ctx = (
                tc.For_i(0, reps, 1) if reps > 1 else contextlib.nullcontext()
            )
            with rep_ctx:
                # alpha -> [1,1] -> broadcast to [128,1]; +a/2 and -a/2.
                a1 = const.tile([1, 1], F32)
                nc.sync.dma_start(out=a1[:, :], in_=al_d.ap()[:, :])
                ab = const.tile([P, 1], F32)
                nc.gpsimd.partition_broadcast(ab[:, :], a1[:, :])
                half = const.tile([P, 1], F32)
                nc.vector.tensor_scalar_mul(half[:, :], ab[:, :], 0.5)
                neghalf = const.tile([P, 1], F32)
                nc.vector.tensor_scalar_mul(neghalf[:, :], ab[:, :], -0.5)

                # x^T shard resident in SBUF, bf16, xg column groups.
                xres = [
                    xres_pool.tile([P, ko, xw], BF16, tag=f"xres{g}",
                                   name=f"xres{g}")
                    for g in range(xg)
                ]
                for g in range(xg):
                    for k4 in range(ko // 4):
                        st = stage.tile([P, 4, xw], F32, tag="xstage")
                        nc.sync.dma_start(
                            out=st[:, :, :],
                            in_=xt_d.ap()[:, k4 * 4:(k4 + 1) * 4,
                                          g * xw:(g + 1) * xw],
                        )
                        nc.vector.tensor_copy(
                            xres[g][:, k4 * 4:(k4 + 1) * 4, :], st[:, :, :]
                        )

                mg = xw // P  # m-groups per xres tile

                for n in range(n_tiles):
                    # Stream + ternarize this n-tile's W^T panel.
                    wq_chunks = []
                    for c in range(ko // 4):
                        st = stage.tile([P, 4, N_TILE], F32, tag="wstage")
                        nc.sync.dma_start(
                            out=st[:, :, :],
                            in_=wt_d.ap()[:, c * 4:(c + 1) * 4,
                                          n * N_TILE:(n + 1) * N_TILE],
                        )
                        q = wqp.tile([P, 4, N_TILE], BF16, tag="wq", name="q")
                        s2 = s2p.tile([P, 4, N_TILE], BF16, tag="s2", name="s2")
                        nc.scalar.sign(q[:, :, :], st[:, :, :], bias=half[:, :])
                        nc.scalar.sign(s2[:, :, :], st[:, :, :],
                                       bias=neghalf[:, :])
                        nc.vector.tensor_tensor(
                            q[:, :, :], q[:, :, :], s2[:, :, :],
                            mybir.AluOpType.add,
                        )
                        wq_chunks.append(q)

                    for m in range(m_sub):
                        g, col = divmod(m, mg)
                        ps = psum.tile([P, N_TILE], F32, tag="ps", name="ps")
                        for k in range(ko):
                            nc.tensor.matmul(
                                ps[:, :],
                                lhsT=xres[g][:, k, col * P:(col + 1) * P],
                                rhs=wq_chunks[k // 4][:, k % 4, :],
                                start=(k == 0),
                                stop=(k == ko - 1),
                            )
                        ot = outs.tile([P, N_TILE], F32, tag="ot", name="ot")
                        # out = psum * (alpha/2): undoes the {-2,0,2}
                        # doubling and applies the alpha weight scale.
                        nc.scalar.mul(ot[:, :], ps[:, :], half[:, :])
                        nc.sync.dma_start(
                            out=out_d.ap()[:, m, n * N_TILE:(n + 1) * N_TILE],
                            in_=ot[:, :],
                        )

    nc.compile()
    return nc


_NC_CACHE = {}


def _get_nc():
    if "nc" not in _NC_CACHE:
        _NC_CACHE["nc"] = build()
    return _NC_CACHE["nc"]


def make_in_maps(x, W, alpha):
    x = np.ascontiguousarray(np.asarray(x, np.float32)).reshape(M_TOT, D_IN)
    W = np.ascontiguousarray(np.asarray(W, np.float32))
    a = np.full((1, 1), np.float32(np.asarray(alpha)), np.float32)
    # wt[p, k, n] = W[n, k*128 + p]
    wt = np.ascontiguousarray(W.reshape(D_OUT, KO, P).transpose(2, 1, 0))
    in_maps = []
    for c in range(N_CORES):
        xs = x[c * M_SHARD:(c + 1) * M_SHARD]
        # xt[p, k, m] = xs[m, k*128 + p]
        xt = np.ascontiguousarray(xs.reshape(M_SHARD, KO, P).transpose(2, 1, 0))
        in_maps.append({"xt": xt, "wt": wt, "alpha": a})
    return in_maps


def gather_out(results):
    outs = []
    for c in range(N_CORES):
        o = results[c]["out"]  # [P, M_SUB, D_OUT]; row = mo*128 + p
        outs.append(o.transpose(1, 0, 2).reshape(M_SHARD, D_OUT))
    return np.concatenate(outs, axis=0).reshape(4, 2048, D_OUT)


def kernel(x, W, alpha):
    nc = _get_nc()
    in_maps = make_in_maps(x, W, alpha)
    res = run_bass_kernel_spmd(nc, in_maps, core_ids=list(range(N_CORES)))
    return gather_out(res.results)


# revision 6
# speedup vs baseline: 1.1734x; 1.1734x over previous
"""BitNetLinear forward on 8 TRN2 NeuronCores.

out = x @ (alpha * clip(round(W/alpha), -1, 1))^T
  x [4, 2048, 4096] f32, W [4096, 4096] f32, alpha scalar f32.

Strategy: data-parallel over the 8192 x-rows (1024 rows/core), W replicated.
No collectives. Host side only reshapes/slices (layout); all arithmetic
(ternary quantization + matmul + alpha scaling) runs on device.

Device kernel (per core):
  - x^T shard resident in SBUF as bf16 (4 tiles of [128, 32, 256]).
  - W^T streamed in [128, 4, 512] f32 chunks; ternarized on the fly via
    T' = Sign(w + a/2) + Sign(w - a/2) in {-2, 0, 2}  (bit-exact vs
    clip(round(w/a)) up to measure-zero ties; validated on HW).
  - 2048 matmuls (bf16, FD=512) accumulate into 8 rotating PSUM banks.
  - PSUM evicted through ScalarE with scale = alpha/2 (folds both the
    ternary doubling and the alpha weight scale), DMA to out.
"""

import contextlib
import sys

if "/opt/trn_rl_repo" not in sys.path:
    sys.path.insert(0, "/opt/trn_rl_repo")

import numpy as np

import concourse.bass as bass  # noqa: F401
import concourse.mybir as mybir
import concourse.tile as tile
from concourse import bacc
from concourse.bass_utils import run_bass_kernel_spmd

P = 128
N_CORES = 8
D_IN = 4096  # contraction
D_OUT = 4096
M_TOT = 4 * 2048
M_SHARD = M_TOT // N_CORES  # 1024
KO = D_IN // P  # 32 k-tiles
N_TILE = 512
XG = 4  # x resident tiles (m column groups)

F32 = mybir.dt.float32
BF16 = mybir.dt.bfloat16


def build(m_shard=M_SHARD, d_in=D_IN, d_out=D_OUT, reps=1):
    ko = d_in // P
    n_tiles = d_out // N_TILE
    m_sub = m_shard // P
    xg = min(XG, max(1, m_shard // 256))
    xw = m_shard // xg

    nc = bacc.Bacc("TRN2", target_bir_lowering=False, debug=False,
                   num_devices=N_CORES)
    xt_d = nc.declare_dram_parameter("xt", [P, ko, m_shard], F32, isOutput=False)
    wt_d = nc.declare_dram_parameter("wt", [P, ko, d_out], F32, isOutput=False)
    al_d = nc.declare_dram_parameter("alpha", [1, 1], F32, isOutput=False)
    out_d = nc.declare_dram_parameter("out", [P, m_sub, d_out], F32, isOutput=True)

    with tile.TileContext(nc) as tc:
        with (
            tc.tile_pool(name="const", bufs=1) as const,
            tc.tile_pool(name="xres", bufs=1) as xres_pool,
            tc.tile_pool(name="stage", bufs=4) as stage,
            tc.tile_pool(name="wq", bufs=2) as wqp,
            tc.tile_pool(name="s2", bufs=2) as s2p,
            tc.tile_pool(name="outs", bufs=4) as outs,
            tc.tile_pool(name="psum", bufs=8, space="PSUM") as psum,
        ):
            rep_ctx = (
                tc.For_i(0, reps, 1) if reps > 1 else contextlib.nullcontext()
            )
            with rep_ctx:
                # alpha -> [1,1] -> broadcast to [128,1]; +a/2 and -a/2.
                a1 = const.tile([1, 1], F32)
                nc.sync.dma_start(out=a1[:, :], in_=al_d.ap()[:, :])
                ab = const.tile([P, 1], F32)
                nc.gpsimd.partition_broadcast(ab[:, :], a1[:, :])
                half = const.tile([P, 1], F32)
                nc.vector.tensor_scalar_mul(half[:, :], ab[:, :], 0.5)
                neghalf = const.tile([P, 1], F32)
                nc.vector.tensor_scalar_mul(neghalf[:, :], ab[:, :], -0.5)

                # x^T shard resident in SBUF, bf16, xg column groups.
                xres = [
                    xres_pool.tile([P, ko, xw], BF16, tag=f"xres{g}",
                                   name=f"xres{g}")
                    for g in range(xg)
                ]
                for g in range(xg):
                    for k4 in range(ko // 4):
                        st = stage.tile([P, 4, xw], F32, tag="xstage")
                        nc.sync.dma_start(
                            out=st[:, :, :],
                            in_=xt_d.ap()[:, k4 * 4:(k4 + 1) * 4,
                                          g * xw:(g + 1) * xw],
                        )
                        nc.vector.tensor_copy(
                            xres[g][:, k4 * 4:(k4 + 1) * 4, :], st[:, :, :]
                        )

                mg = xw // P  # m-groups per xres tile

                for n in range(n_tiles):
                    # Stream + ternarize this n-tile's W^T panel into ONE
                    # tile: matmuls then carry a single wait per n-tile
                    # instead of one per chunk.
                    wq = wqp.tile([P, ko, N_TILE], BF16, tag="wq", name="wq")
                    for c in range(ko // 4):
                        st = stage.tile([P, 4, N_TILE], F32, tag="wstage")
                        nc.sync.dma_start(
                            out=st[:, :, :],
                            in_=wt_d.ap()[:, c * 4:(c + 1) * 4,
                                          n * N_TILE:(n + 1) * N_TILE],
                        )
                        qs = wq[:, c * 4:(c + 1) * 4, :]
                        s2 = s2p.tile([P, 4, N_TILE], BF16, tag="s2", name="s2")
                        nc.scalar.sign(qs, st[:, :, :], bias=half[:, :])
                        nc.scalar.sign(s2[:, :, :], st[:, :, :],
                                       bias=neghalf[:, :])
                        nc.vector.tensor_tensor(
                            qs, qs, s2[:, :, :], mybir.AluOpType.add
                        )

                    for m in range(m_sub):
                        g, col = divmod(m, mg)
                        ps = psum.tile([P, N_TILE], F32, tag="ps", name="ps")
                        for k in range(ko):
                            nc.tensor.matmul(
                                ps[:, :],
                                lhsT=xres[g][:, k, col * P:(col + 1) * P],
                                rhs=wq[:, k, :],
                                start=(k == 0),
                                stop=(k == ko - 1),
                            )
                        ot = outs.tile([P, N_TILE], F32, tag="ot", name="ot")
                        # out = psum * (alpha/2): undoes the {-2,0,2}
                        # doubling and applies the alpha weight scale.
                        nc.vector.tensor_scalar_mul(ot[:, :], ps[:, :],
                                                    half[:, :])
                        nc.sync.dma_start(
                            out=out_d.ap()[:, m, n * N_TILE:(n + 1) * N_TILE],
                            in_=ot[:, :],
                        )

    nc.compile()
    return nc


_NC_CACHE = {}


def _get_nc():
    if "nc" not in _NC_CACHE:
        _NC_CACHE["nc"] = build()
    return _NC_CACHE["nc"]


def make_in_maps(x, W, alpha):
    x = np.ascontiguousarray(np.asarray(x, np.float32)).reshape(M_TOT, D_IN)
    W = np.ascontiguousarray(np.asarray(W, np.float32))
    a = np.full((1, 1), np.float32(np.asarray(alpha)), np.float32)
    # wt[p, k, n] = W[n, k*128 + p]
    wt = np.ascontiguousarray(W.reshape(D_OUT, KO, P).transpose(2, 1, 0))
    in_maps = []
    for c in range(N_CORES):
        xs = x[c * M_SHARD:(c + 1) * M_SHARD]
        # xt[p, k, m] = xs[m, k*128 + p]
        xt = np.ascontiguousarray(xs.reshape(M_SHARD, KO, P).transpose(2, 1, 0))
        in_maps.append({"xt": xt, "wt": wt, "alpha": a})
    return in_maps


def gather_out(results):
    outs = []
    for c in range(N_CORES):
        o = results[c]["out"]  # [P, M_SUB, D_OUT]; row = mo*128 + p
        outs.append(o.transpose(1, 0, 2).reshape(M_SHARD, D_OUT))
    return np.concatenate(outs, axis=0).reshape(4, 2048, D_OUT)


def kernel(x, W, alpha):
    nc = _get_nc()
    in_maps = make_in_maps(x, W, alpha)
    res = run_bass_kernel_spmd(nc, in_maps, core_ids=list(range(N_CORES)))
    return gather_out(res.results)
